# revision 27
# baseline (speedup 1.0000x reference)
"""Span-attention kernel for Trainium2 (8 NeuronCores, SPMD).

Strategy
--------
Data-parallel over bsz: core b owns batch row b (bsz == 8 == n_cores).
Host routes each query q to core qb[q] and packs queries (both span
sets mixed) into T query tiles of 128.  Each tile j has a FIXED window
of wt[j] (<=3) aligned 128-token tiles starting at tile a[j]; a query
with token span [s, e] fits tile j iff [s>>7, e>>7] is inside the
window.  Windows are uniform across cores (SPMD), assignment of
queries to tiles is per-core (greedy earliest-expiring-window).

Per-core device program (everything bf16/fp8 on the wire):
  1. enc_ext[2048, 257] = X_b @ [W | W @ attn_w]  (PE bf16, streamed in
     8 token chunks so the first matmul starts ~2us in).
     ACT: E = exp(logit col); EncE[t,:] = [enc[t,:]*E[t] | E[t]] (bf16,
     E col written by DVE).
  2. out[q, 0:257] = sum_w maskT_w[t, q] * EncE[a_j+w][t, :]  (PE,
     fp8 mask weights from host x bf16 EncE).  col 256 = softmax
     denominator.  DVE copies PSUM -> SBUF bf16; DMA out.
Host divides by the denominator column and scatters rows back.
"""

import os
import sys

import numpy as np
import ml_dtypes

sys.path.insert(0, "/opt/trn_rl_repo")

from contextlib import ExitStack

from concourse import bass, bacc, mybir
import concourse.tile as tile
from concourse.bass_utils import run_bass_kernel_spmd

P = 128
BSZ = 8
SEQ = 2048
HD = 1024
PD = 256
NCOL = PD + 1    # value cols + logit/denominator col
MT = SEQ // P    # 16 token tiles
KT = HD // P     # 8 contraction tiles
CH = [3, 3, 2, 2, 2, 2, 2]          # chunk sizes in 128-token tiles
CHOFF = [0, 3, 6, 8, 10, 12, 14]    # cumulative
NCHUNK = len(CH)
Q = 8192
LAG = 3
BF16 = ml_dtypes.bfloat16
FP8 = ml_dtypes.float8_e4m3

_cache = {}


def _wts(a):
    return [min(3, MT - aj) for aj in a]


def _build_program(T, a, wt, exp_bias, use_bias):
    nc = bacc.Bacc("TRN2", target_bir_lowering=False)
    f32 = mybir.dt.float32
    bf16 = mybir.dt.bfloat16
    fp8 = mybir.dt.float8e4

    NW = sum(wt)
    moff = np.concatenate([[0], np.cumsum(wt)])[:-1]
    last = [a[j] + wt[j] - 1 for j in range(T)]
    bins_by_last = {}
    for j in range(T):
        bins_by_last.setdefault(last[j], []).append(j)

    # device-layout params (host pre-permutes so every DMA is plain 2D
    # with large contiguous rows -> few descriptors, fast HWDGE gen)
    xT = nc.declare_dram_parameter("xT", [P, KT * SEQ], bf16, isOutput=False)
    wext = nc.declare_dram_parameter("wext", [P, KT * NCOL], bf16,
                                     isOutput=False)
    masks = nc.declare_dram_parameter("masks", [P, NW * P], fp8, isOutput=False)
    if use_bias:
        bex = nc.declare_dram_parameter("bex", [1, NCOL], f32, isOutput=False)
        ones1 = nc.declare_dram_parameter("ones1", [1, P], f32, isOutput=False)
    res = nc.declare_dram_parameter("res", [T * P, NCOL], bf16, isOutput=True)

    with tile.TileContext(nc) as tc, ExitStack() as ctx:
        const_pool = ctx.enter_context(tc.tile_pool(name="const", bufs=1))
        xt_pool = ctx.enter_context(tc.tile_pool(name="xt", bufs=1))
        ence_pool = ctx.enter_context(tc.tile_pool(name="ence", bufs=1))
        ecol_pool = ctx.enter_context(tc.tile_pool(name="ecol", bufs=4))
        out_pool = ctx.enter_context(tc.tile_pool(name="out", bufs=3))
        ps_enc = ctx.enter_context(tc.tile_pool(name="ps_enc", bufs=4, space="PSUM"))
        ps_out = ctx.enter_context(tc.tile_pool(name="ps_out", bufs=3, space="PSUM"))

        # ---- PE prewarm: ramp the clock before real data lands ----
        dummy_sb = const_pool.tile([P, 512], bf16, tag="dummy")
        nc.gpsimd.memset(dummy_sb[:], 0)
        warm_ps = ps_out.tile([P, 512], f32, tag="warm", bufs=1)
        for _ in range(15):
            nc.tensor.matmul(warm_ps[:], lhsT=dummy_sb[:, 0:P],
                             rhs=dummy_sb[:], start=True, stop=True)

        # ---- loads: single SP stream in dependency order: wext/chunk0
        # halves, chunk1, chunk2, masks_a, chunk3.., masks_b.  Nothing
        # else competes for wire bandwidth in the critical window. ----
        wext_sb = const_pool.tile([P, KT * NCOL], bf16, tag="wext")
        chunks = [xt_pool.tile([P, KT * CH[c] * P], bf16, tag=f"xt{c}",
                               name=f"xt{c}")
                  for c in range(NCHUNK)]
        masks_sb = const_pool.tile([P, NW * P], fp8, tag="masks")
        NWH = NW // 2

        def load_chunk(c):
            off = KT * CHOFF[c] * P
            nc.sync.dma_start(chunks[c][:],
                              xT[:, off:off + KT * CH[c] * P])

        HL = KT * NCOL // 2
        CL = KT * CH[0] * P // 2
        nc.sync.dma_start(wext_sb[:, 0:HL], wext[:, 0:HL])
        nc.scalar.dma_start(chunks[0][:, 0:CL], xT[:, 0:CL])
        nc.sync.dma_start(wext_sb[:, HL:], wext[:, HL:])
        nc.sync.dma_start(chunks[0][:, CL:], xT[:, CL:2 * CL])
        load_chunk(1)
        nc.scalar.dma_start(masks_sb[:, 0:NWH * P], masks[:, 0:NWH * P])
        off2 = KT * CHOFF[2] * P
        nc.scalar.dma_start(chunks[2][:], xT[:, off2:off2 + KT * CH[2] * P])
        for c in range(3, NCHUNK):
            load_chunk(c)
        nc.sync.dma_start(masks_sb[:, NWH * P:], masks[:, NWH * P:])
        if use_bias:
            bex_sb = const_pool.tile([1, NCOL], f32, tag="bex")
            nc.scalar.dma_start(bex_sb[:], bex[:])
            ones1_sb = const_pool.tile([1, P], f32, tag="ones1")
            nc.scalar.dma_start(ones1_sb[:], ones1[:])

        ence_tiles = []
        # res write groups: quads for the bulk, singles for the last 4
        # tiles (each write starts as soon as its cast lands)
        grp_of = {}
        groups = []
        i0 = 0
        while i0 < T - 4:
            g = min(4, T - 4 - i0)
            groups.append((i0, g))
            i0 += g
        for i1 in range(max(0, T - 4), T):
            groups.append((i1, 1))
        for gi, (st, sz) in enumerate(groups):
            for ii in range(st, st + sz):
                grp_of[ii] = (gi, st, sz)
        res_stage = [None]
        ndone = [0]

        def emit_bin(j):
            i = ndone[0]
            ndone[0] += 1
            assert i == j  # emission order must match host res-row layout
            out_ps = ps_out.tile([P, NCOL], f32, tag="out")
            for w in range(wt[j]):
                nc.tensor.matmul(
                    out_ps[:],
                    lhsT=masks_sb[:, (moff[j] + w) * P:(moff[j] + w + 1) * P],
                    rhs=ence_tiles[a[j] + w][:],
                    start=(w == 0), stop=(w == wt[j] - 1))
            gi, st, sz = grp_of[i]
            if i == st:
                res_stage[0] = out_pool.tile([P, sz * NCOL], bf16,
                                             tag=f"res{sz}",
                                             bufs=(4 if sz == 1 else 3),
                                             name=f"res_sb{i}")
            res_sb = res_stage[0]
            o = i - st
            nc.vector.tensor_copy(res_sb[:, o * NCOL:(o + 1) * NCOL], out_ps[:])
            if i == st + sz - 1:
                if sz == 1:
                    (nc.scalar if i % 2 == 0 else nc.sync).dma_start(
                        res[i * P:(i + 1) * P, :], res_sb[:])
                else:
                    dst = res[st * P:(st + sz) * P, :].rearrange(
                        "(h p) c -> p h c", h=sz)
                    src = res_sb[:].rearrange("p (h c) -> p h c", h=sz)
                    nc.sync.dma_start(dst, src)

        m2c = [(c, o) for c in range(NCHUNK) for o in range(CH[c])]
        for m in range(MT):
            c, o = m2c[m]
            ctok = CH[c] * P
            enc_ps = ps_enc.tile([P, NCOL], f32, tag="enc")
            for k in range(KT):
                nc.tensor.matmul(
                    enc_ps[:],
                    lhsT=chunks[c][:, k * ctok + o * P:k * ctok + (o + 1) * P],
                    rhs=wext_sb[:, k * NCOL:(k + 1) * NCOL],
                    start=(k == 0), stop=(k == KT - 1 and not use_bias))
            if use_bias:
                nc.tensor.matmul(enc_ps[:], lhsT=ones1_sb[:], rhs=bex_sb[:],
                                 start=False, stop=True)
            ecol = ecol_pool.tile([P, 1], f32, tag="ecol")
            nc.scalar.activation(ecol[:], enc_ps[:, PD:PD + 1],
                                 mybir.ActivationFunctionType.Exp,
                                 bias=float(exp_bias))
            ence = ence_pool.tile([P, NCOL], bf16, tag=f"ence{m}")
            nc.scalar.activation(ence[:, 0:PD], enc_ps[:, 0:PD],
                                 mybir.ActivationFunctionType.Copy,
                                 scale=ecol[:])
            nc.vector.tensor_copy(ence[:, PD:PD + 1], ecol[:])
            ence_tiles.append(ence)
            for j in bins_by_last.get(m - LAG, []):
                emit_bin(j)
        for mm in range(MT - LAG, MT):
            for j in bins_by_last.get(mm, []):
                emit_bin(j)

    nc.compile()
    return nc


def _assign(kk, ke, a, wt):
    """Greedily pack queries (interval [kk, ke] of token tiles) into
    len(a) bins of 128 slots; bin j accepts iff its window covers the
    interval.  Returns (per-bin index lists, None) or (None, fail_k)."""
    T = len(a)
    elig = {}
    for k0 in range(MT):
        for k1 in (k0, k0 + 1):
            if k1 >= MT:
                continue
            lst = [j for j in range(T)
                   if a[j] <= k0 and k1 <= a[j] + wt[j] - 1]
            lst.sort(key=lambda j: (a[j] + wt[j], a[j]))
            elig[(k0, k1)] = lst
    order = np.lexsort((-kk, ke))
    cap = [P] * T
    bins = [[] for _ in range(T)]
    for idx in order:
        for j in elig.get((kk[idx], ke[idx]), []):
            if cap[j] > 0:
                cap[j] -= 1
                bins[j].append(idx)
                break
        else:
            return None, int(kk[idx])
    return bins, None


def _prep(inputs):
    enc = np.asarray(inputs["encoded_input"], np.float32)
    proj_w = np.asarray(inputs["proj_w"], np.float32)
    proj_b = np.asarray(inputs["proj_b"], np.float32)
    attn_w = np.asarray(inputs["attn_w"], np.float32)
    attn_b = float(np.asarray(inputs["attn_b"], np.float32))
    qb = np.asarray(inputs["query_batch_idx"], np.int64)

    use_bias = bool(np.any(proj_b != 0.0))
    waw = (proj_w @ attn_w)[:, None].astype(np.float32)
    # device layout: [P, KT, NCOL] flattened (contiguous per partition row)
    wext = np.ascontiguousarray(
        np.concatenate([proj_w, waw], axis=1).reshape(KT, P, NCOL)
        .transpose(1, 0, 2).reshape(P, KT * NCOL)).astype(BF16)
    exp_bias = attn_b + (0.0 if use_bias else float(proj_b @ attn_w))
    bex = np.zeros((1, NCOL), np.float32)
    bex[0, :PD] = proj_b
    bex[0, PD] = float(proj_b @ attn_w)

    s_all, e_all = [], []
    for ss in (1, 2):
        s = np.asarray(inputs[f"start_ids_{ss}"], np.int64)
        e = np.asarray(inputs[f"end_ids_{ss}"], np.int64)
        e = np.maximum(e, s)  # setup_inputs guarantees e >= s
        s_all.append(s)
        e_all.append(e)
    # combined query stream per core: (set, orig index, s, e)
    s_cat = np.concatenate(s_all)
    e_cat = np.concatenate(e_all)
    ss_cat = np.concatenate([np.zeros(Q, np.int64), np.ones(Q, np.int64)])
    qi_cat = np.concatenate([np.arange(Q), np.arange(Q)])
    qb_cat = np.concatenate([qb, qb])
    kk_cat = (s_cat >> 7).astype(np.int64)
    ke_cat = (e_cat >> 7).astype(np.int64)

    per_core_sel = [np.nonzero(qb_cat == b)[0] for b in range(BSZ)]

    # one bin per window position, then add bins where packing fails
    a = list(range(MT - 2 + 1))  # a = 0..14
    while True:
        wt = _wts(a)
        all_bins = []
        fail = None
        for b in range(BSZ):
            sel = per_core_sel[b]
            bins, fail = _assign(kk_cat[sel], ke_cat[sel], a, wt)
            if bins is None:
                break
            all_bins.append([sel[idx] for idx in bins])
        if fail is None:
            break
        a = sorted(a + [min(fail, MT - 2)])
        assert len(a) <= 32, "query packing failed"
    T = len(a)
    wt = _wts(a)

    NW = sum(wt)
    moff = np.concatenate([[0], np.cumsum(wt)])[:-1]
    in_maps = []
    rowmaps = []
    for b in range(BSZ):
        blob = np.zeros((P, NW * P), np.float32)
        out_ss = np.full(T * P, -1, np.int64)
        out_qi = np.full(T * P, -1, np.int64)
        for j in range(T):
            g = np.asarray(all_bins[b][j], np.int64)
            n = len(g)
            if n == 0:
                continue
            srel = (s_cat[g] - (a[j] << 7)).astype(np.int64)
            erel = (e_cat[g] - (a[j] << 7)).astype(np.int64)
            D = np.zeros((wt[j] * P + 1, P), np.float32)
            D[srel, np.arange(n)] = 1.0
            np.subtract.at(D, (erel + 1, np.arange(n)), 1.0)
            M = np.cumsum(D[:-1], axis=0)
            blob[:, moff[j] * P:(moff[j] + wt[j]) * P] = (
                M.reshape(wt[j], P, P).transpose(1, 0, 2).reshape(P, wt[j] * P))
            out_ss[j * P:j * P + n] = ss_cat[g]
            out_qi[j * P:j * P + n] = qi_cat[g]
        # device layout: per-chunk blocks [P, KT, tok_c] flattened, concat
        x = enc[b].T
        blocks = []
        for t0, sz in zip(CHOFF, CH):
            blocks.append(
                x[:, t0 * P:(t0 + sz) * P].reshape(KT, P, sz * P)
                .transpose(1, 0, 2).reshape(P, KT * sz * P))
        xT_b = np.ascontiguousarray(np.concatenate(blocks, axis=1)).astype(BF16)
        im = {"xT": xT_b, "wext": wext, "masks": blob.astype(FP8)}
        if use_bias:
            im["bex"] = bex
            im["ones1"] = np.ones((1, P), np.float32)
        in_maps.append(im)
        rowmaps.append((out_ss, out_qi))
    return T, a, wt, in_maps, rowmaps, exp_bias, use_bias


def kernel(**inputs):
    T, a, wt, in_maps, rowmaps, exp_bias, use_bias = _prep(inputs)
    key = (T, tuple(a), exp_bias, use_bias)
    if key not in _cache:
        _cache[key] = _build_program(T, a, wt, exp_bias, use_bias)
    nc = _cache[key]
    r = run_bass_kernel_spmd(nc, in_maps, core_ids=list(range(BSZ)),
                             trace=bool(int(os.environ.get("KTRACE", "0"))))
    res1 = np.zeros((Q, PD), np.float32)
    res2 = np.zeros((Q, PD), np.float32)
    for b in range(BSZ):
        rb = np.asarray(r.results[b]["res"], np.float32)
        out_ss, out_qi = rowmaps[b]
        valid = out_qi >= 0
        vals = rb[valid, :PD]
        den = rb[valid, PD]
        den[den == 0] = 1.0
        vals = vals / den[:, None]
        vss = out_ss[valid]
        vqi = out_qi[valid]
        res1[vqi[vss == 0]] = vals[vss == 0]
        res2[vqi[vss == 1]] = vals[vss == 1]
    kernel.last_exec_ns = r.exec_time_ns
    return res1, res2


# revision 29
# speedup vs baseline: 1.0201x; 1.0201x over previous
"""Span-attention kernel for Trainium2 (8 NeuronCores, SPMD).

Strategy
--------
Data-parallel over bsz: core b owns batch row b (bsz == 8 == n_cores).
Host routes each query q to core qb[q] and packs queries (both span
sets mixed) into T query tiles of 128.  Each tile j has a FIXED window
of wt[j] (<=3) aligned 128-token tiles starting at tile a[j]; a query
with token span [s, e] fits tile j iff [s>>7, e>>7] is inside the
window.  Windows are uniform across cores (SPMD), assignment of
queries to tiles is per-core (greedy earliest-expiring-window).

Per-core device program (everything bf16/fp8 on the wire):
  1. enc_ext[2048, 257] = X_b @ [W | W @ attn_w]  (PE bf16, streamed in
     8 token chunks so the first matmul starts ~2us in).
     ACT: E = exp(logit col); EncE[t,:] = [enc[t,:]*E[t] | E[t]] (bf16,
     E col written by DVE).
  2. out[q, 0:257] = sum_w maskT_w[t, q] * EncE[a_j+w][t, :]  (PE,
     fp8 mask weights from host x bf16 EncE).  col 256 = softmax
     denominator.  DVE copies PSUM -> SBUF bf16; DMA out.
Host divides by the denominator column and scatters rows back.
"""

import os
import sys

import numpy as np
import ml_dtypes

sys.path.insert(0, "/opt/trn_rl_repo")

from contextlib import ExitStack

from concourse import bass, bacc, mybir
import concourse.tile as tile
from concourse.bass_utils import run_bass_kernel_spmd

P = 128
BSZ = 8
SEQ = 2048
HD = 1024
PD = 256
NCOL = PD + 1    # value cols + logit/denominator col
MT = SEQ // P    # 16 token tiles
KT = HD // P     # 8 contraction tiles
CH = [3, 3, 2, 2, 2, 2, 2]          # chunk sizes in 128-token tiles
CHOFF = [0, 3, 6, 8, 10, 12, 14]    # cumulative
NCHUNK = len(CH)
Q = 8192
LAG = 3
BF16 = ml_dtypes.bfloat16
FP8 = ml_dtypes.float8_e4m3

_cache = {}


def _wts(a):
    return [min(3, MT - aj) for aj in a]


def _build_program(T, a, wt, exp_bias, use_bias):
    nc = bacc.Bacc("TRN2", target_bir_lowering=False)
    f32 = mybir.dt.float32
    bf16 = mybir.dt.bfloat16
    fp8 = mybir.dt.float8e4

    NW = sum(wt)
    moff = np.concatenate([[0], np.cumsum(wt)])[:-1]
    last = [a[j] + wt[j] - 1 for j in range(T)]
    bins_by_last = {}
    for j in range(T):
        bins_by_last.setdefault(last[j], []).append(j)

    # device-layout params (host pre-permutes so every DMA is plain 2D
    # with large contiguous rows -> few descriptors, fast HWDGE gen)
    xT = nc.declare_dram_parameter("xT", [P, KT * SEQ], bf16, isOutput=False)
    wext = nc.declare_dram_parameter("wext", [P, KT * NCOL], bf16,
                                     isOutput=False)
    masks = nc.declare_dram_parameter("masks", [P, NW * P], fp8, isOutput=False)
    if use_bias:
        bex = nc.declare_dram_parameter("bex", [1, NCOL], f32, isOutput=False)
        ones1 = nc.declare_dram_parameter("ones1", [1, P], f32, isOutput=False)
    res = nc.declare_dram_parameter("res", [T * P, NCOL], bf16, isOutput=True)

    with tile.TileContext(nc) as tc, ExitStack() as ctx:
        const_pool = ctx.enter_context(tc.tile_pool(name="const", bufs=1))
        xt_pool = ctx.enter_context(tc.tile_pool(name="xt", bufs=1))
        ence_pool = ctx.enter_context(tc.tile_pool(name="ence", bufs=1))
        ecol_pool = ctx.enter_context(tc.tile_pool(name="ecol", bufs=4))
        out_pool = ctx.enter_context(tc.tile_pool(name="out", bufs=3))
        ps_enc = ctx.enter_context(tc.tile_pool(name="ps_enc", bufs=4, space="PSUM"))
        ps_out = ctx.enter_context(tc.tile_pool(name="ps_out", bufs=3, space="PSUM"))

        # ---- PE prewarm: ramp the clock before real data lands ----
        dummy_sb = const_pool.tile([P, 512], bf16, tag="dummy")
        nc.gpsimd.memset(dummy_sb[:], 0)
        warm_ps = ps_out.tile([P, 512], f32, tag="warm", bufs=1)
        for _ in range(15):
            nc.tensor.matmul(warm_ps[:], lhsT=dummy_sb[:, 0:P],
                             rhs=dummy_sb[:], start=True, stop=True)

        # ---- loads: single SP stream in dependency order: wext/chunk0
        # halves, chunk1, chunk2, masks_a, chunk3.., masks_b.  Nothing
        # else competes for wire bandwidth in the critical window. ----
        wext_sb = const_pool.tile([P, KT * NCOL], bf16, tag="wext")
        chunks = [xt_pool.tile([P, KT * CH[c] * P], bf16, tag=f"xt{c}",
                               name=f"xt{c}")
                  for c in range(NCHUNK)]
        masks_sb = const_pool.tile([P, NW * P], fp8, tag="masks")
        NWH = NW // 2

        def load_chunk(c):
            off = KT * CHOFF[c] * P
            nc.sync.dma_start(chunks[c][:],
                              xT[:, off:off + KT * CH[c] * P])

        HL = KT * NCOL // 2
        CL = KT * CH[0] * P // 2
        nc.sync.dma_start(wext_sb[:, 0:HL], wext[:, 0:HL])
        nc.sync.dma_start(chunks[0][:, 0:CL], xT[:, 0:CL])
        nc.sync.dma_start(wext_sb[:, HL:], wext[:, HL:])
        nc.sync.dma_start(chunks[0][:, CL:], xT[:, CL:2 * CL])
        load_chunk(1)
        load_chunk(2)
        nc.sync.dma_start(masks_sb[:, 0:NWH * P], masks[:, 0:NWH * P])
        for c in range(3, NCHUNK):
            load_chunk(c)
        nc.sync.dma_start(masks_sb[:, NWH * P:], masks[:, NWH * P:])
        if use_bias:
            bex_sb = const_pool.tile([1, NCOL], f32, tag="bex")
            nc.scalar.dma_start(bex_sb[:], bex[:])
            ones1_sb = const_pool.tile([1, P], f32, tag="ones1")
            nc.scalar.dma_start(ones1_sb[:], ones1[:])

        ence_tiles = []
        # res write groups: quads for the bulk, singles for the last 4
        # tiles (each write starts as soon as its cast lands)
        grp_of = {}
        groups = []
        i0 = 0
        while i0 < T - 4:
            g = min(4, T - 4 - i0)
            groups.append((i0, g))
            i0 += g
        for i1 in range(max(0, T - 4), T):
            groups.append((i1, 1))
        for gi, (st, sz) in enumerate(groups):
            for ii in range(st, st + sz):
                grp_of[ii] = (gi, st, sz)
        res_stage = [None]
        ndone = [0]

        def emit_bin(j):
            i = ndone[0]
            ndone[0] += 1
            assert i == j  # emission order must match host res-row layout
            out_ps = ps_out.tile([P, NCOL], f32, tag="out")
            for w in range(wt[j]):
                nc.tensor.matmul(
                    out_ps[:],
                    lhsT=masks_sb[:, (moff[j] + w) * P:(moff[j] + w + 1) * P],
                    rhs=ence_tiles[a[j] + w][:],
                    start=(w == 0), stop=(w == wt[j] - 1))
            gi, st, sz = grp_of[i]
            if i == st:
                res_stage[0] = out_pool.tile([P, sz * NCOL], bf16,
                                             tag=f"res{sz}",
                                             bufs=(4 if sz == 1 else 3),
                                             name=f"res_sb{i}")
            res_sb = res_stage[0]
            o = i - st
            nc.vector.tensor_copy(res_sb[:, o * NCOL:(o + 1) * NCOL], out_ps[:])
            if i == st + sz - 1:
                if sz == 1:
                    (nc.scalar if i % 2 == 0 else nc.sync).dma_start(
                        res[i * P:(i + 1) * P, :], res_sb[:])
                else:
                    dst = res[st * P:(st + sz) * P, :].rearrange(
                        "(h p) c -> p h c", h=sz)
                    src = res_sb[:].rearrange("p (h c) -> p h c", h=sz)
                    nc.sync.dma_start(dst, src)

        m2c = [(c, o) for c in range(NCHUNK) for o in range(CH[c])]
        for m in range(MT):
            c, o = m2c[m]
            ctok = CH[c] * P
            enc_ps = ps_enc.tile([P, NCOL], f32, tag="enc")
            for k in range(KT):
                nc.tensor.matmul(
                    enc_ps[:],
                    lhsT=chunks[c][:, k * ctok + o * P:k * ctok + (o + 1) * P],
                    rhs=wext_sb[:, k * NCOL:(k + 1) * NCOL],
                    start=(k == 0), stop=(k == KT - 1 and not use_bias))
            if use_bias:
                nc.tensor.matmul(enc_ps[:], lhsT=ones1_sb[:], rhs=bex_sb[:],
                                 start=False, stop=True)
            ecol = ecol_pool.tile([P, 1], f32, tag="ecol")
            nc.scalar.activation(ecol[:], enc_ps[:, PD:PD + 1],
                                 mybir.ActivationFunctionType.Exp,
                                 bias=float(exp_bias))
            ence = ence_pool.tile([P, NCOL], bf16, tag=f"ence{m}")
            nc.scalar.activation(ence[:, 0:PD], enc_ps[:, 0:PD],
                                 mybir.ActivationFunctionType.Copy,
                                 scale=ecol[:])
            nc.vector.tensor_copy(ence[:, PD:PD + 1], ecol[:])
            ence_tiles.append(ence)
            for j in bins_by_last.get(m - LAG, []):
                emit_bin(j)
        for mm in range(MT - LAG, MT):
            for j in bins_by_last.get(mm, []):
                emit_bin(j)

    nc.compile()
    return nc


def _assign(kk, ke, a, wt):
    """Greedily pack queries (interval [kk, ke] of token tiles) into
    len(a) bins of 128 slots; bin j accepts iff its window covers the
    interval.  Returns (per-bin index lists, None) or (None, fail_k)."""
    T = len(a)
    elig = {}
    for k0 in range(MT):
        for k1 in (k0, k0 + 1):
            if k1 >= MT:
                continue
            lst = [j for j in range(T)
                   if a[j] <= k0 and k1 <= a[j] + wt[j] - 1]
            lst.sort(key=lambda j: (a[j] + wt[j], a[j]))
            elig[(k0, k1)] = lst
    order = np.lexsort((-kk, ke))
    cap = [P] * T
    bins = [[] for _ in range(T)]
    for idx in order:
        for j in elig.get((kk[idx], ke[idx]), []):
            if cap[j] > 0:
                cap[j] -= 1
                bins[j].append(idx)
                break
        else:
            return None, int(kk[idx])
    return bins, None


def _prep(inputs):
    enc = np.asarray(inputs["encoded_input"], np.float32)
    proj_w = np.asarray(inputs["proj_w"], np.float32)
    proj_b = np.asarray(inputs["proj_b"], np.float32)
    attn_w = np.asarray(inputs["attn_w"], np.float32)
    attn_b = float(np.asarray(inputs["attn_b"], np.float32))
    qb = np.asarray(inputs["query_batch_idx"], np.int64)

    use_bias = bool(np.any(proj_b != 0.0))
    waw = (proj_w @ attn_w)[:, None].astype(np.float32)
    # device layout: [P, KT, NCOL] flattened (contiguous per partition row)
    wext = np.ascontiguousarray(
        np.concatenate([proj_w, waw], axis=1).reshape(KT, P, NCOL)
        .transpose(1, 0, 2).reshape(P, KT * NCOL)).astype(BF16)
    exp_bias = attn_b + (0.0 if use_bias else float(proj_b @ attn_w))
    bex = np.zeros((1, NCOL), np.float32)
    bex[0, :PD] = proj_b
    bex[0, PD] = float(proj_b @ attn_w)

    s_all, e_all = [], []
    for ss in (1, 2):
        s = np.asarray(inputs[f"start_ids_{ss}"], np.int64)
        e = np.asarray(inputs[f"end_ids_{ss}"], np.int64)
        e = np.maximum(e, s)  # setup_inputs guarantees e >= s
        s_all.append(s)
        e_all.append(e)
    # combined query stream per core: (set, orig index, s, e)
    s_cat = np.concatenate(s_all)
    e_cat = np.concatenate(e_all)
    ss_cat = np.concatenate([np.zeros(Q, np.int64), np.ones(Q, np.int64)])
    qi_cat = np.concatenate([np.arange(Q), np.arange(Q)])
    qb_cat = np.concatenate([qb, qb])
    kk_cat = (s_cat >> 7).astype(np.int64)
    ke_cat = (e_cat >> 7).astype(np.int64)

    per_core_sel = [np.nonzero(qb_cat == b)[0] for b in range(BSZ)]

    # one bin per window position, then add bins where packing fails
    a = list(range(MT - 2 + 1))  # a = 0..14
    while True:
        wt = _wts(a)
        all_bins = []
        fail = None
        for b in range(BSZ):
            sel = per_core_sel[b]
            bins, fail = _assign(kk_cat[sel], ke_cat[sel], a, wt)
            if bins is None:
                break
            all_bins.append([sel[idx] for idx in bins])
        if fail is None:
            break
        a = sorted(a + [min(fail, MT - 2)])
        assert len(a) <= 32, "query packing failed"
    T = len(a)
    # effective window width per bin: widest actual content across cores
    # (uniform across cores; usually 2, not the nominal 3)
    wt = [1] * T
    for b in range(BSZ):
        for j in range(T):
            g = all_bins[b][j]
            if g:
                w = int(max(ke_cat[i] for i in g)) - a[j] + 1
                wt[j] = max(wt[j], w)

    NW = sum(wt)
    moff = np.concatenate([[0], np.cumsum(wt)])[:-1]
    in_maps = []
    rowmaps = []
    for b in range(BSZ):
        blob = np.zeros((P, NW * P), np.float32)
        out_ss = np.full(T * P, -1, np.int64)
        out_qi = np.full(T * P, -1, np.int64)
        for j in range(T):
            g = np.asarray(all_bins[b][j], np.int64)
            n = len(g)
            if n == 0:
                continue
            srel = (s_cat[g] - (a[j] << 7)).astype(np.int64)
            erel = (e_cat[g] - (a[j] << 7)).astype(np.int64)
            D = np.zeros((wt[j] * P + 1, P), np.float32)
            D[srel, np.arange(n)] = 1.0
            np.subtract.at(D, (erel + 1, np.arange(n)), 1.0)
            M = np.cumsum(D[:-1], axis=0)
            blob[:, moff[j] * P:(moff[j] + wt[j]) * P] = (
                M.reshape(wt[j], P, P).transpose(1, 0, 2).reshape(P, wt[j] * P))
            out_ss[j * P:j * P + n] = ss_cat[g]
            out_qi[j * P:j * P + n] = qi_cat[g]
        # device layout: per-chunk blocks [P, KT, tok_c] flattened, concat
        x = enc[b].T
        blocks = []
        for t0, sz in zip(CHOFF, CH):
            blocks.append(
                x[:, t0 * P:(t0 + sz) * P].reshape(KT, P, sz * P)
                .transpose(1, 0, 2).reshape(P, KT * sz * P))
        xT_b = np.ascontiguousarray(np.concatenate(blocks, axis=1)).astype(BF16)
        im = {"xT": xT_b, "wext": wext, "masks": blob.astype(FP8)}
        if use_bias:
            im["bex"] = bex
            im["ones1"] = np.ones((1, P), np.float32)
        in_maps.append(im)
        rowmaps.append((out_ss, out_qi))
    return T, a, wt, in_maps, rowmaps, exp_bias, use_bias


def kernel(**inputs):
    T, a, wt, in_maps, rowmaps, exp_bias, use_bias = _prep(inputs)
    key = (T, tuple(a), exp_bias, use_bias)
    if key not in _cache:
        _cache[key] = _build_program(T, a, wt, exp_bias, use_bias)
    nc = _cache[key]
    r = run_bass_kernel_spmd(nc, in_maps, core_ids=list(range(BSZ)),
                             trace=bool(int(os.environ.get("KTRACE", "0"))))
    res1 = np.zeros((Q, PD), np.float32)
    res2 = np.zeros((Q, PD), np.float32)
    for b in range(BSZ):
        rb = np.asarray(r.results[b]["res"], np.float32)
        out_ss, out_qi = rowmaps[b]
        valid = out_qi >= 0
        vals = rb[valid, :PD]
        den = rb[valid, PD]
        den[den == 0] = 1.0
        vals = vals / den[:, None]
        vss = out_ss[valid]
        vqi = out_qi[valid]
        res1[vqi[vss == 0]] = vals[vss == 0]
        res2[vqi[vss == 1]] = vals[vss == 1]
    kernel.last_exec_ns = r.exec_time_ns
    return res1, res2


# revision 35
# speedup vs baseline: 1.1916x; 1.1681x over previous
"""Span-attention kernel for Trainium2 (8 NeuronCores, SPMD).

Strategy
--------
Data-parallel over bsz: core b owns batch row b (bsz == 8 == n_cores).
Host routes each query q to core qb[q] and packs queries (both span
sets mixed) into T query tiles of 128.  Each tile j has a FIXED window
of wt[j] (<=3) aligned 128-token tiles starting at tile a[j]; a query
with token span [s, e] fits tile j iff [s>>7, e>>7] is inside the
window.  Windows are uniform across cores (SPMD), assignment of
queries to tiles is per-core (greedy earliest-expiring-window).

Per-core device program (everything bf16/fp8 on the wire):
  1. enc_ext[2048, 257] = X_b @ [W | W @ attn_w]  (PE bf16, streamed in
     8 token chunks so the first matmul starts ~2us in).
     ACT: E = exp(logit col); EncE[t,:] = [enc[t,:]*E[t] | E[t]] (bf16,
     E col written by DVE).
  2. out[q, 0:257] = sum_w maskT_w[t, q] * EncE[a_j+w][t, :]  (PE,
     fp8 mask weights from host x bf16 EncE).  col 256 = softmax
     denominator.  DVE copies PSUM -> SBUF bf16; DMA out.
Host divides by the denominator column and scatters rows back.
"""

import os
import sys

import numpy as np
import ml_dtypes

sys.path.insert(0, "/opt/trn_rl_repo")

from contextlib import ExitStack

from concourse import bass, bacc, mybir
import concourse.tile as tile
from concourse.bass_utils import run_bass_kernel_spmd

P = 128
BSZ = 8
SEQ = 2048
HD = 1024
PD = 256
NCOL = PD + 1    # value cols + logit/denominator col
MT = SEQ // P    # 16 token tiles
KT = HD // P     # 8 contraction tiles
CH = [3, 3, 2, 2, 2, 2, 2]          # chunk sizes in 128-token tiles
CHOFF = [0, 3, 6, 8, 10, 12, 14]    # cumulative
NCHUNK = len(CH)
Q = 8192
LAG = 3
BF16 = ml_dtypes.bfloat16
FP8 = ml_dtypes.float8_e4m3

_cache = {}


def _wts(a):
    return [min(3, MT - aj) for aj in a]


def _build_program(T, a, wt, exp_bias, use_bias):
    nc = bacc.Bacc("TRN2", target_bir_lowering=False)
    f32 = mybir.dt.float32
    bf16 = mybir.dt.bfloat16
    fp8 = mybir.dt.float8e4

    NW = sum(wt)
    moff = np.concatenate([[0], np.cumsum(wt)])[:-1]
    last = [a[j] + wt[j] - 1 for j in range(T)]
    bins_by_last = {}
    for j in range(T):
        bins_by_last.setdefault(last[j], []).append(j)

    # device-layout params (host pre-permutes so every DMA is plain 2D
    # with large contiguous rows -> few descriptors, fast HWDGE gen)
    xT = nc.declare_dram_parameter("xT", [P, KT * SEQ], bf16, isOutput=False)
    wext = nc.declare_dram_parameter("wext", [P, KT * NCOL], bf16,
                                     isOutput=False)
    masks = nc.declare_dram_parameter("masks", [P, NW * P], fp8, isOutput=False)
    if use_bias:
        bex = nc.declare_dram_parameter("bex", [1, NCOL], f32, isOutput=False)
        ones1 = nc.declare_dram_parameter("ones1", [1, P], f32, isOutput=False)
    res = nc.declare_dram_parameter("res", [T * P, NCOL], bf16, isOutput=True)

    with tile.TileContext(nc) as tc, ExitStack() as ctx:
        const_pool = ctx.enter_context(tc.tile_pool(name="const", bufs=1))
        xt_pool = ctx.enter_context(tc.tile_pool(name="xt", bufs=1))
        ence_pool = ctx.enter_context(tc.tile_pool(name="ence", bufs=1))
        ecol_pool = ctx.enter_context(tc.tile_pool(name="ecol", bufs=4))
        out_pool = ctx.enter_context(tc.tile_pool(name="out", bufs=3))
        ps_enc = ctx.enter_context(tc.tile_pool(name="ps_enc", bufs=4, space="PSUM"))
        ps_out = ctx.enter_context(tc.tile_pool(name="ps_out", bufs=3, space="PSUM"))

        # ---- PE prewarm: ramp the clock before real data lands ----
        dummy_sb = const_pool.tile([P, 512], bf16, tag="dummy")
        nc.gpsimd.memset(dummy_sb[:], 0)
        warm_ps = ps_out.tile([P, 512], f32, tag="warm", bufs=1)
        for _ in range(15):
            nc.tensor.matmul(warm_ps[:], lhsT=dummy_sb[:, 0:P],
                             rhs=dummy_sb[:], start=True, stop=True)

        # ---- loads: single SP stream in dependency order: wext/chunk0
        # halves, chunk1, chunk2, masks_a, chunk3.., masks_b.  Nothing
        # else competes for wire bandwidth in the critical window. ----
        wext_sb = const_pool.tile([P, KT * NCOL], bf16, tag="wext")
        chunks = [xt_pool.tile([P, KT * CH[c] * P], bf16, tag=f"xt{c}",
                               name=f"xt{c}")
                  for c in range(NCHUNK)]
        masks_sb = const_pool.tile([P, NW * P], fp8, tag="masks")
        NWH = NW // 2

        def load_chunk(c):
            off = KT * CHOFF[c] * P
            nc.sync.dma_start(chunks[c][:],
                              xT[:, off:off + KT * CH[c] * P])

        HL = KT * NCOL // 2
        CL = KT * CH[0] * P // 2
        nc.sync.dma_start(wext_sb[:, 0:HL], wext[:, 0:HL])
        nc.sync.dma_start(chunks[0][:, 0:CL], xT[:, 0:CL])
        nc.sync.dma_start(wext_sb[:, HL:], wext[:, HL:])
        nc.sync.dma_start(chunks[0][:, CL:], xT[:, CL:2 * CL])
        load_chunk(1)
        load_chunk(2)
        nc.sync.dma_start(masks_sb[:, 0:NWH * P], masks[:, 0:NWH * P])
        for c in range(3, NCHUNK):
            load_chunk(c)
        nc.sync.dma_start(masks_sb[:, NWH * P:], masks[:, NWH * P:])
        if use_bias:
            bex_sb = const_pool.tile([1, NCOL], f32, tag="bex")
            nc.scalar.dma_start(bex_sb[:], bex[:])
            ones1_sb = const_pool.tile([1, P], f32, tag="ones1")
            nc.scalar.dma_start(ones1_sb[:], ones1[:])

        ence_tiles = []
        # res write groups: quads for the bulk, singles for the last 4
        # tiles (each write starts as soon as its cast lands)
        grp_of = {}
        groups = []
        i0 = 0
        while i0 < T - 4:
            g = min(4, T - 4 - i0)
            groups.append((i0, g))
            i0 += g
        for i1 in range(max(0, T - 4), T):
            groups.append((i1, 1))
        for gi, (st, sz) in enumerate(groups):
            for ii in range(st, st + sz):
                grp_of[ii] = (gi, st, sz)
        res_stage = {}

        def emit_bin(j):
            out_ps = ps_out.tile([P, NCOL], f32, tag="out")
            for w in range(wt[j]):
                nc.tensor.matmul(
                    out_ps[:],
                    lhsT=masks_sb[:, (moff[j] + w) * P:(moff[j] + w + 1) * P],
                    rhs=ence_tiles[a[j] + w][:],
                    start=(w == 0), stop=(w == wt[j] - 1))
            gi, st, sz = grp_of[j]
            if gi not in res_stage:
                res_stage[gi] = [out_pool.tile([P, sz * NCOL], bf16,
                                               tag=f"res{sz}",
                                               bufs=(4 if sz == 1 else 3),
                                               name=f"res_sb{st}"), 0]
            ent = res_stage[gi]
            res_sb = ent[0]
            o = j - st
            nc.vector.tensor_copy(res_sb[:, o * NCOL:(o + 1) * NCOL], out_ps[:])
            ent[1] += 1
            if ent[1] == sz:
                if sz == 1:
                    (nc.scalar if j % 2 == 0 else nc.sync).dma_start(
                        res[j * P:(j + 1) * P, :], res_sb[:])
                else:
                    dst = res[st * P:(st + sz) * P, :].rearrange(
                        "(h p) c -> p h c", h=sz)
                    src = res_sb[:].rearrange("p (h c) -> p h c", h=sz)
                    nc.sync.dma_start(dst, src)

        m2c = [(c, o) for c in range(NCHUNK) for o in range(CH[c])]
        for m in range(MT):
            c, o = m2c[m]
            ctok = CH[c] * P
            enc_ps = ps_enc.tile([P, NCOL], f32, tag="enc")
            for k in range(KT):
                nc.tensor.matmul(
                    enc_ps[:],
                    lhsT=chunks[c][:, k * ctok + o * P:k * ctok + (o + 1) * P],
                    rhs=wext_sb[:, k * NCOL:(k + 1) * NCOL],
                    start=(k == 0), stop=(k == KT - 1 and not use_bias))
            if use_bias:
                nc.tensor.matmul(enc_ps[:], lhsT=ones1_sb[:], rhs=bex_sb[:],
                                 start=False, stop=True)
            ecol = ecol_pool.tile([P, 1], f32, tag="ecol")
            nc.scalar.activation(ecol[:], enc_ps[:, PD:PD + 1],
                                 mybir.ActivationFunctionType.Exp,
                                 bias=float(exp_bias))
            ence = ence_pool.tile([P, NCOL], bf16, tag=f"ence{m}")
            nc.scalar.activation(ence[:, 0:PD], enc_ps[:, 0:PD],
                                 mybir.ActivationFunctionType.Copy,
                                 scale=ecol[:])
            nc.vector.tensor_copy(ence[:, PD:PD + 1], ecol[:])
            ence_tiles.append(ence)
            for j in bins_by_last.get(m - LAG, []):
                emit_bin(j)
        for mm in range(MT - LAG, MT):
            for j in bins_by_last.get(mm, []):
                emit_bin(j)

    nc.compile()
    return nc


def _assign(kk, ke, a, wt):
    """Greedily pack queries (interval [kk, ke] of token tiles) into
    len(a) bins of 128 slots; bin j accepts iff its window covers the
    interval.  Returns (per-bin index lists, None) or (None, fail_k)."""
    T = len(a)
    elig = {}
    for k0 in range(MT):
        for k1 in (k0, k0 + 1):
            if k1 >= MT:
                continue
            lst = [j for j in range(T)
                   if a[j] <= k0 and k1 <= a[j] + wt[j] - 1]
            lst.sort(key=lambda j: (a[j] + wt[j], a[j]))
            elig[(k0, k1)] = lst
    order = np.lexsort((-kk, ke))
    cap = [P] * T
    bins = [[] for _ in range(T)]
    for idx in order:
        for j in elig.get((kk[idx], ke[idx]), []):
            if cap[j] > 0:
                cap[j] -= 1
                bins[j].append(idx)
                break
        else:
            return None, int(kk[idx])
    return bins, None


def _prep(inputs):
    enc = np.asarray(inputs["encoded_input"], np.float32)
    proj_w = np.asarray(inputs["proj_w"], np.float32)
    proj_b = np.asarray(inputs["proj_b"], np.float32)
    attn_w = np.asarray(inputs["attn_w"], np.float32)
    attn_b = float(np.asarray(inputs["attn_b"], np.float32))
    qb = np.asarray(inputs["query_batch_idx"], np.int64)

    use_bias = bool(np.any(proj_b != 0.0))
    waw = (proj_w @ attn_w)[:, None].astype(np.float32)
    # device layout: [P, KT, NCOL] flattened (contiguous per partition row)
    wext = np.ascontiguousarray(
        np.concatenate([proj_w, waw], axis=1).reshape(KT, P, NCOL)
        .transpose(1, 0, 2).reshape(P, KT * NCOL)).astype(BF16)
    exp_bias = attn_b + (0.0 if use_bias else float(proj_b @ attn_w))
    bex = np.zeros((1, NCOL), np.float32)
    bex[0, :PD] = proj_b
    bex[0, PD] = float(proj_b @ attn_w)

    s_all, e_all = [], []
    for ss in (1, 2):
        s = np.asarray(inputs[f"start_ids_{ss}"], np.int64)
        e = np.asarray(inputs[f"end_ids_{ss}"], np.int64)
        e = np.maximum(e, s)  # setup_inputs guarantees e >= s
        s_all.append(s)
        e_all.append(e)
    # combined query stream per core: (set, orig index, s, e)
    s_cat = np.concatenate(s_all)
    e_cat = np.concatenate(e_all)
    ss_cat = np.concatenate([np.zeros(Q, np.int64), np.ones(Q, np.int64)])
    qi_cat = np.concatenate([np.arange(Q), np.arange(Q)])
    qb_cat = np.concatenate([qb, qb])
    kk_cat = (s_cat >> 7).astype(np.int64)
    ke_cat = (e_cat >> 7).astype(np.int64)

    per_core_sel = [np.nonzero(qb_cat == b)[0] for b in range(BSZ)]

    # one bin per window position, then add bins where packing fails
    a = list(range(MT - 2 + 1))  # a = 0..14
    while True:
        wt = _wts(a)
        all_bins = []
        fail = None
        for b in range(BSZ):
            sel = per_core_sel[b]
            bins, fail = _assign(kk_cat[sel], ke_cat[sel], a, wt)
            if bins is None:
                break
            all_bins.append([sel[idx] for idx in bins])
        if fail is None:
            break
        a = sorted(a + [min(fail, MT - 2)])
        assert len(a) <= 32, "query packing failed"
    T = len(a)
    # effective window width per bin: widest actual content across cores
    # (uniform across cores; usually 2, not the nominal 3)
    wt = [1] * T
    for b in range(BSZ):
        for j in range(T):
            g = all_bins[b][j]
            if len(g):
                w = int(ke_cat[np.asarray(g)].max()) - a[j] + 1
                wt[j] = max(wt[j], w)

    NW = sum(wt)
    moff = np.concatenate([[0], np.cumsum(wt)])[:-1]
    in_maps = []
    rowmaps = []
    for b in range(BSZ):
        blob = np.zeros((P, NW * P), np.float32)
        out_ss = np.full(T * P, -1, np.int64)
        out_qi = np.full(T * P, -1, np.int64)
        for j in range(T):
            g = np.asarray(all_bins[b][j], np.int64)
            n = len(g)
            if n == 0:
                continue
            srel = (s_cat[g] - (a[j] << 7)).astype(np.int64)
            erel = (e_cat[g] - (a[j] << 7)).astype(np.int64)
            D = np.zeros((wt[j] * P + 1, P), np.float32)
            D[srel, np.arange(n)] = 1.0
            np.subtract.at(D, (erel + 1, np.arange(n)), 1.0)
            M = np.cumsum(D[:-1], axis=0)
            blob[:, moff[j] * P:(moff[j] + wt[j]) * P] = (
                M.reshape(wt[j], P, P).transpose(1, 0, 2).reshape(P, wt[j] * P))
            out_ss[j * P:j * P + n] = ss_cat[g]
            out_qi[j * P:j * P + n] = qi_cat[g]
        # device layout: per-chunk blocks [P, KT, tok_c] flattened, concat
        x = enc[b].T
        blocks = []
        for t0, sz in zip(CHOFF, CH):
            blocks.append(
                x[:, t0 * P:(t0 + sz) * P].reshape(KT, P, sz * P)
                .transpose(1, 0, 2).reshape(P, KT * sz * P))
        xT_b = np.ascontiguousarray(np.concatenate(blocks, axis=1)).astype(BF16)
        im = {"xT": xT_b, "wext": wext, "masks": blob.astype(FP8)}
        if use_bias:
            im["bex"] = bex
            im["ones1"] = np.ones((1, P), np.float32)
        in_maps.append(im)
        rowmaps.append((out_ss, out_qi))
    return T, a, wt, in_maps, rowmaps, exp_bias, use_bias


def kernel(**inputs):
    T, a, wt, in_maps, rowmaps, exp_bias, use_bias = _prep(inputs)
    key = (T, tuple(a), tuple(wt), exp_bias, use_bias)
    if key not in _cache:
        _cache[key] = _build_program(T, a, wt, exp_bias, use_bias)
    nc = _cache[key]
    r = run_bass_kernel_spmd(nc, in_maps, core_ids=list(range(BSZ)),
                             trace=bool(int(os.environ.get("KTRACE", "0"))))
    res1 = np.zeros((Q, PD), np.float32)
    res2 = np.zeros((Q, PD), np.float32)
    for b in range(BSZ):
        rb = np.asarray(r.results[b]["res"], np.float32)
        out_ss, out_qi = rowmaps[b]
        valid = out_qi >= 0
        vals = rb[valid, :PD]
        den = rb[valid, PD]
        den[den == 0] = 1.0
        vals = vals / den[:, None]
        vss = out_ss[valid]
        vqi = out_qi[valid]
        res1[vqi[vss == 0]] = vals[vss == 0]
        res2[vqi[vss == 1]] = vals[vss == 1]
    kernel.last_exec_ns = r.exec_time_ns
    return res1, res2
